# revision 2
# baseline (speedup 1.0000x reference)
"""Trainium2 kernel for nn_HadamardRotation: y = x @ H, H = 4096x4096 Walsh-Hadamard.

Strategy
--------
H4096 = H64 (x) H64 (Kronecker). Writing d = 64*hi + lo, e = 64*hi' + lo':

    y[r, e] = sum_{hi,lo} H64[lo,lo'] * H64[hi,hi'] * x[r, d]

Two matmul stages with 128-wide contraction (block-diagonal I2 (x) H64
weights), separated by an on-chip "corner turn" (SBUF->SBUF DMA partition
shuffle), all operating in the transposed domain (d on partitions, rows on
the free axis). Host does the cheap transposes / index unscrambles; the
device only ever issues contiguous >=1KB DMA lines.

FLOPs: 2 * 128/4096 of the naive matmul = 16x reduction.

Data parallel over 8 cores: rows sharded 16384 -> 8 x 2048, weights
replicated.

Layouts (per core, R = 2048 rows):
  xt  DRAM in  (32, 128, R): xt[a, 64*mu+lo, r] = x[r, 128*a + 64*mu + lo]
  B1  (128,128): B1[64*mu+lo, 2*lo'+mu]     = H64[lo, lo']
  B2  (128,128): B2[64*nu+32*mu+a, 2*hi'+nu] = H64[2*a+mu, hi']
  stage A (chunk a): u_a[p, r] = sum_k B1[k, p] xt[a, k, r]
      => u_a[4c + (2*nu+mu)] holds (hi = 2a+mu, lo' = 2c+nu)
  corner turn:  v_c[32*t + a, r] = u_a[4*c + t, r]
  stage B (chunk c): Y[c, m, r] = sum_q B2[q, m] v_c[q, r]
      => Y[c, 2*hi'+nu, r] = y[r, 64*hi' + 2*c + nu]
"""

import math
import numpy as np
import ml_dtypes

import concourse.bass as bass
import concourse.mybir as mybir
import concourse.tile as tile
from concourse import bacc
from concourse.bass_utils import run_bass_kernel_spmd

N_CORES = 8
DIM = 4096
R_TOTAL = 4 * 4096          # rows after flattening (4, 4096, DIM)
R = R_TOTAL // N_CORES      # rows per core
N = 512                     # free-dim slab (one PSUM bank of fp32)
SLABS = R // N

# dtype mode: "fp32" (exact, PE 4 cyc/row), "fp32r" (fp32 storage, fast PE
# mode), "bf16" (half storage+DMA for x/intermediate, exact weights)
MODE = "bf16"

# tuning knobs (overridable for benching)
CFG = dict(
    ycopy="vector2",   # engine for psum->sbuf copy of stage-B out: vector|any|vector2 (split DVE/ACT)
    ucopy="vector",    # engine for psum->sbuf copy of stage-A out
    turn_eng="scalar",  # corner-turn DMA engine: scalar|sync|gpsimd|rr (round robin)
    in_eng="sync",
    out_eng="sync",
    in_batch=4,        # chunks per input DMA
    out_batch=4,       # batch output DMAs over this many c-chunks
    turn_slabs=1,      # how many N-slabs share one corner-turn DMA
    pipeline=1,        # emit stage A of slab s+1 before stage B of slab s
    ycast=0,           # stage-B out staged as bf16 in SBUF, SWDGE casts to f32
    xbufs=3, ubufs=2, vbufs=4, ybufs=4,
)


def _walsh_hadamard64():
    h = np.array([[1.0]], dtype=np.float64)
    while h.shape[0] < 64:
        h = np.block([[h, h], [h, -h]]) / math.sqrt(2.0)
    return h.astype(np.float32)


def _build_weights(H64):
    B1 = np.zeros((128, 128), dtype=np.float32)
    b1v = B1.reshape(2, 64, 64, 2)
    for mu in range(2):
        b1v[mu, :, :, mu] = H64
    B2 = np.zeros((128, 128), dtype=np.float32)
    b2v = B2.reshape(2, 2, 32, 64, 2)
    for nu in range(2):
        for mu in range(2):
            b2v[nu, mu, :, :, nu] = H64[mu::2, :]
    return B1, B2


_NC_CACHE = {}


def _build_bass(mode, loop=0, cfg=None):
    cfg = dict(CFG, **(cfg or {}))
    key = (mode, loop, tuple(sorted(cfg.items())))
    if key in _NC_CACHE:
        return _NC_CACHE[key]

    f32 = mybir.dt.float32
    dt_in = mybir.dt.bfloat16 if mode == "bf16" else f32
    mm_cast = (lambda ap: ap.bitcast(mybir.dt.float32r)) if mode == "fp32r" else (lambda ap: ap)

    nc = bacc.Bacc("TRN2", target_bir_lowering=False, debug=False,
                   num_devices=N_CORES)
    xt_d = nc.dram_tensor("xt", [32, 128, R], dt_in, kind="ExternalInput")
    B1_d = nc.dram_tensor("B1", [128, 128], dt_in, kind="ExternalInput")
    B2_d = nc.dram_tensor("B2", [128, 128], dt_in, kind="ExternalInput")
    Y_d = nc.dram_tensor("Y", [32, 128, R], f32, kind="ExternalOutput")

    OB = cfg["out_batch"]

    with tile.TileContext(nc) as tc:
        with (
            tc.tile_pool(name="wpool", bufs=1) as wpool,
            tc.tile_pool(name="xpool", bufs=cfg["xbufs"]) as xpool,
            tc.tile_pool(name="upool", bufs=cfg["ubufs"]) as upool,
            tc.tile_pool(name="vpool", bufs=cfg["vbufs"]) as vpool,
            tc.tile_pool(name="ypool", bufs=cfg["ybufs"]) as ypool,
            tc.tile_pool(name="psA", bufs=4, space="PSUM") as psA,
            tc.tile_pool(name="psB", bufs=4, space="PSUM") as psB,
        ):
            B1_sb = wpool.tile([128, 128], dt_in)
            nc.sync.dma_start(B1_sb[:], B1_d[:])
            B2_sb = wpool.tile([128, 128], dt_in)
            nc.sync.dma_start(B2_sb[:], B2_d[:])

            in_eng = getattr(nc, cfg["in_eng"])
            out_eng = getattr(nc, cfg["out_eng"])
            turn_eng = None if cfg["turn_eng"] == "rr" else getattr(nc, cfg["turn_eng"])

            def copy(engine, dst, src, i):
                if engine == "vector":
                    nc.vector.tensor_copy(dst, src)
                elif engine == "vector2":
                    # alternate DVE / ACT so neither engine binds
                    if i % 2 == 0:
                        nc.vector.tensor_copy(dst, src)
                    else:
                        nc.any.tensor_copy(dst, src)
                else:
                    nc.any.tensor_copy(dst, src)

            turn_rr = [nc.scalar, nc.sync, nc.gpsimd]

            def turn(i):
                if cfg["turn_eng"] == "rr":
                    return turn_rr[i % 3]
                return turn_eng

            TS = cfg["turn_slabs"]
            IB = cfg["in_batch"]

            def phaseA(sg):
                    u_all = upool.tile([128, 32, TS * N], dt_in)
                    for ts in range(TS):
                        s = sg * TS + ts
                        ns = slice(s * N, (s + 1) * N)
                        for g in range(32 // IB):
                            xg = xpool.tile([128, IB, N], dt_in)
                            in_eng.dma_start(
                                xg[:],
                                xt_d[IB * g:IB * (g + 1), :, ns].transpose([1, 0, 2]))
                            for j in range(IB):
                                a = IB * g + j
                                pu = psA.tile([128, N], f32)
                                nc.tensor.matmul(pu[:], mm_cast(B1_sb[:]),
                                                 mm_cast(xg[:, j, :]),
                                                 start=True, stop=True)
                                copy(cfg["ucopy"],
                                     u_all[:, a, ts * N:(ts + 1) * N], pu[:], a)
                    return u_all

            def phaseB(sg, u_all):
                    # corner turn + stage B
                    ut = u_all.tensor
                    PU = u_all.ap[0][0]  # partition stride in elements
                    L = TS * N
                    dt_y = mybir.dt.bfloat16 if cfg["ycast"] else f32
                    y_eng = nc.gpsimd if cfg["ycast"] else out_eng
                    for cb in range(32 // OB):
                        ybs = [ypool.tile([128, OB, N], dt_y, name=f"yb{ts}")
                               for ts in range(TS)]
                        for j in range(OB):
                            c = cb * OB + j
                            vc = vpool.tile([128, L], dt_in)
                            in_ap = bass.AP(ut, 4 * c * PU,
                                            [[PU, 4], [L, 32], [1, L]])
                            turn(c).dma_start(vc[:], in_ap)
                            for ts in range(TS):
                                py = psB.tile([128, N], f32)
                                nc.tensor.matmul(py[:], mm_cast(B2_sb[:]),
                                                 mm_cast(vc[:, ts * N:(ts + 1) * N]),
                                                 start=True, stop=True)
                                copy(cfg["ycopy"], ybs[ts][:, j, :], py[:], c + ts)
                        for ts in range(TS):
                            s = sg * TS + ts
                            y_eng.dma_start(
                                Y_d[cb * OB:(cb + 1) * OB, :,
                                    s * N:(s + 1) * N].transpose([1, 0, 2]),
                                ybs[ts][:])

            def body():
                if cfg["pipeline"]:
                    # software pipeline: emit stage A of slab-group sg+1
                    # before stage B of sg, so PE never stalls on the turn.
                    pending = None
                    for sg in range(SLABS // TS):
                        u_all = phaseA(sg)
                        if pending is not None:
                            phaseB(*pending)
                        pending = (sg, u_all)
                    phaseB(*pending)
                else:
                    for sg in range(SLABS // TS):
                        phaseB(sg, phaseA(sg))

            if loop:
                with tc.For_i(0, loop, 1):
                    body()
            else:
                body()

    nc.compile()
    _NC_CACHE[key] = nc
    return nc


def _prep_inputs(x, H, mode):
    np_in = ml_dtypes.bfloat16 if mode == "bf16" else np.float32
    H64 = (np.asarray(H, dtype=np.float32)[::64, ::64] * 8.0).astype(np.float32)
    B1, B2 = _build_weights(H64)
    B1 = B1.astype(np_in)
    B2 = B2.astype(np_in)
    xf = np.asarray(x, dtype=np.float32).reshape(R_TOTAL, DIM)
    in_maps = []
    for i in range(N_CORES):
        shard = xf[i * R:(i + 1) * R]                     # (R, DIM)
        xt = np.ascontiguousarray(shard.T, dtype=np_in)   # (DIM, R)
        xt = xt.reshape(32, 128, R)
        in_maps.append({"xt": xt, "B1": B1, "B2": B2})
    return in_maps


def _unscramble(results):
    outs = []
    for i in range(N_CORES):
        Y = results[i]["Y"]                               # (32, 128, R) f32
        y = Y.reshape(32, 64, 2, R).transpose(3, 1, 0, 2).reshape(R, DIM)
        outs.append(y)
    return np.concatenate(outs, axis=0).reshape(4, 4096, DIM).astype(np.float32)


def kernel(x, H, _trace=False):
    nc = _build_bass(MODE)
    in_maps = _prep_inputs(x, H, MODE)
    res = run_bass_kernel_spmd(nc, in_maps, core_ids=list(range(N_CORES)),
                               trace=_trace)
    out = _unscramble(res.results)
    if _trace:
        return out, res
    return out



# revision 5
# speedup vs baseline: 1.0544x; 1.0544x over previous
"""Trainium2 kernel for nn_HadamardRotation: y = x @ H, H = 4096x4096 Walsh-Hadamard.

Strategy
--------
H4096 = H64 (x) H64 (Kronecker). Writing d = 64*hi + lo, e = 64*hi' + lo':

    y[r, e] = sum_{hi,lo} H64[lo,lo'] * H64[hi,hi'] * x[r, d]

Two matmul stages with 128-wide contraction (block-diagonal I2 (x) H64
weights), separated by an on-chip "corner turn" (SBUF->SBUF DMA partition
shuffle), all operating in the transposed domain (d on partitions, rows on
the free axis). Host does the cheap transposes / index unscrambles; the
device only ever issues contiguous >=2KB DMA lines.

FLOPs: 2 * 128/4096 of the naive matmul = 16x reduction.

Data parallel over 8 cores: rows sharded 16384 -> 8 x 2048, weights
replicated.

Per-core layouts (R = 2048 rows, L = 1024 slab-group, SG=2 groups,
IB=4 input chunk batch, OB=4 output chunk batch):
  xt DRAM in  [SG*8, 128, IB*L]: mirrors the SBUF xg tiles exactly so each
     input DMA is 128 contiguous 8KB descriptors.
     xt[sg*8+g, q, j*L+rr] = x[r=sg*L+rr, d=128*(4g+j)+q]
  B1 (128,128): B1[64*mu+lo, 2*lo'+mu]      = H64[lo, lo']
  B2 (128,128): B2[64*nu+32*mu+a, 2*hi'+nu] = H64[2*a+mu, hi']
  stage A (chunk a=4g+j): u[p, a, rr] = sum_k B1[k, p] xg[k, j, rr]
      => u[4c + (2*nu+mu), a] holds (hi = 2a+mu, lo' = 2c+nu)
  corner turn (chunk c):  vc[32*t + a, rr] = u[4*c + t, a, rr]
  stage B (chunk c): yb[m, rr] = sum_q B2[q, m] vc[q, rr]
      => yb[2*hi'+nu] = y[rr, 64*hi' + 2*c + nu]  (bf16)
  Y DRAM out [SG*8, 128, OB*L]: mirrors the SBUF yb tiles (128 x 8KB
     descriptors per DMA); host unscrambles + casts to f32.
"""

import math
import numpy as np
import ml_dtypes

import concourse.bass as bass
import concourse.mybir as mybir
import concourse.tile as tile
from concourse import bacc
from concourse.bass_utils import run_bass_kernel_spmd

N_CORES = 8
DIM = 4096
R_TOTAL = 4 * 4096          # rows after flattening (4, 4096, DIM)
R = R_TOTAL // N_CORES      # rows per core
N = 512                     # matmul free-dim slab (one PSUM bank of fp32)
MODE = "bf16"               # storage dtype for x/intermediate/output

# tuning knobs
CFG = dict(
    TS=2,              # slabs per slab-group (L = TS*N)
    IB=4,              # chunks per input DMA / xg tile
    OB=4,              # chunks per output DMA / yb tile
    turn_engs="pool,sync",       # round-robin queues for corner-turn DMAs
    ucopy_engs="vector,scalar",  # stage-A psum->sbuf copy engines (PSUM
                                 # readable by DVE/ACT only, not gpsimd)
    ycopy_engs="vector,scalar",       # stage-B psum->sbuf copy engines
    in_eng="sync",
    out_eng="sync",
    xbufs=2, ubufs=2, vbufs=4, ybufs=2,
)


def _walsh_hadamard64():
    h = np.array([[1.0]], dtype=np.float64)
    while h.shape[0] < 64:
        h = np.block([[h, h], [h, -h]]) / math.sqrt(2.0)
    return h.astype(np.float32)


def _build_weights(H64):
    B1 = np.zeros((128, 128), dtype=np.float32)
    b1v = B1.reshape(2, 64, 64, 2)
    for mu in range(2):
        b1v[mu, :, :, mu] = H64
    B2 = np.zeros((128, 128), dtype=np.float32)
    b2v = B2.reshape(2, 2, 32, 64, 2)
    for nu in range(2):
        for mu in range(2):
            b2v[nu, mu, :, :, nu] = H64[mu::2, :]
    return B1, B2


_NC_CACHE = {}


def _build_bass(cfg=None):
    cfg = dict(CFG, **(cfg or {}))
    key = tuple(sorted(cfg.items()))
    if key in _NC_CACHE:
        return _NC_CACHE[key]

    f32 = mybir.dt.float32
    bf16 = mybir.dt.bfloat16

    TS, IB, OB = cfg["TS"], cfg["IB"], cfg["OB"]
    L = TS * N
    SG = R // L                 # slab groups
    NG = 32 // IB               # input chunk groups per slab group
    NCB = 32 // OB              # output chunk batches per slab group

    nc = bacc.Bacc("TRN2", target_bir_lowering=False, debug=False,
                   num_devices=N_CORES)
    xt_d = nc.dram_tensor("xt", [SG * NG, 128, IB * L], bf16,
                          kind="ExternalInput")
    B1_d = nc.dram_tensor("B1", [128, 128], bf16, kind="ExternalInput")
    B2_d = nc.dram_tensor("B2", [128, 128], bf16, kind="ExternalInput")
    Y_d = nc.dram_tensor("Y", [SG * NCB, 128, OB * L], bf16,
                         kind="ExternalOutput")

    with tile.TileContext(nc) as tc:
        with (
            tc.tile_pool(name="wpool", bufs=1) as wpool,
            tc.tile_pool(name="xpool", bufs=cfg["xbufs"]) as xpool,
            tc.tile_pool(name="upool", bufs=cfg["ubufs"]) as upool,
            tc.tile_pool(name="vpool", bufs=cfg["vbufs"]) as vpool,
            tc.tile_pool(name="ypool", bufs=cfg["ybufs"]) as ypool,
            tc.tile_pool(name="psA", bufs=4, space="PSUM") as psA,
            tc.tile_pool(name="psB", bufs=4, space="PSUM") as psB,
        ):
            B1_sb = wpool.tile([128, 128], bf16)
            nc.sync.dma_start(B1_sb[:], B1_d[:])
            B2_sb = wpool.tile([128, 128], bf16)
            nc.sync.dma_start(B2_sb[:], B2_d[:])

            in_eng = getattr(nc, cfg["in_eng"])
            out_eng = getattr(nc, cfg["out_eng"])

            def eng_list(names):
                out = []
                for nm in names.split(","):
                    nm = nm.strip()
                    out.append(nc.gpsimd if nm == "pool" else getattr(nc, nm))
                return out

            turn_engs = eng_list(cfg["turn_engs"])
            ucopy_engs = eng_list(cfg["ucopy_engs"])
            ycopy_engs = eng_list(cfg["ycopy_engs"])

            def copy(engs, i, dst, src):
                e = engs[i % len(engs)]
                if e is nc.scalar:
                    nc.scalar.copy(dst, src)
                else:
                    e.tensor_copy(dst, src)

            def phaseA(sg):
                u_all = upool.tile([128, 32, L], bf16)
                for g in range(NG):
                    xg = xpool.tile([128, IB, L], bf16)
                    in_eng.dma_start(xg[:], xt_d[sg * NG + g, :, :])
                    for j in range(IB):
                        a = IB * g + j
                        for ts in range(TS):
                            pu = psA.tile([128, N], f32)
                            nc.tensor.matmul(pu[:], B1_sb[:],
                                             xg[:, j, ts * N:(ts + 1) * N],
                                             start=True, stop=True)
                            copy(ucopy_engs, a * TS + ts,
                                 u_all[:, a, ts * N:(ts + 1) * N], pu[:])
                return u_all

            def phaseB(sg, u_all):
                ut = u_all.tensor
                PU = u_all.ap[0][0]  # partition stride in elements
                for cb in range(NCB):
                    yb = ypool.tile([128, OB, L], bf16)
                    for j in range(OB):
                        c = cb * OB + j
                        vc = vpool.tile([128, L], bf16)
                        in_ap = bass.AP(ut, 4 * c * PU,
                                        [[PU, 4], [L, 32], [1, L]])
                        turn_engs[c % len(turn_engs)].dma_start(vc[:], in_ap)
                        for ts in range(TS):
                            py = psB.tile([128, N], f32)
                            nc.tensor.matmul(py[:], B2_sb[:],
                                             vc[:, ts * N:(ts + 1) * N],
                                             start=True, stop=True)
                            copy(ycopy_engs, c * TS + ts,
                                 yb[:, j, ts * N:(ts + 1) * N], py[:])
                    out_eng.dma_start(Y_d[sg * NCB + cb, :, :], yb[:])

            # software pipeline: emit stage A of slab-group sg+1 before
            # stage B of sg, so the PE never stalls on the corner turn.
            pending = None
            for sg in range(SG):
                u_all = phaseA(sg)
                if pending is not None:
                    phaseB(*pending)
                pending = (sg, u_all)
            phaseB(*pending)

    nc.compile()
    _NC_CACHE[key] = nc
    return nc


def _prep_inputs(x, H, cfg=None):
    cfg = dict(CFG, **(cfg or {}))
    TS, IB = cfg["TS"], cfg["IB"]
    L = TS * N
    SG = R // L
    NG = 32 // IB
    H64 = (np.asarray(H, dtype=np.float32)[::64, ::64] * 8.0).astype(np.float32)
    B1, B2 = _build_weights(H64)
    B1 = B1.astype(ml_dtypes.bfloat16)
    B2 = B2.astype(ml_dtypes.bfloat16)
    xf = np.asarray(x, dtype=np.float32).reshape(R_TOTAL, DIM)
    in_maps = []
    for i in range(N_CORES):
        shard = xf[i * R:(i + 1) * R]                     # (R, DIM)
        # [sg, rr, a, q] -> [sg, g, q, j, rr]
        xt = shard.reshape(SG, L, 32, 128)
        xt = xt.transpose(0, 2, 3, 1).reshape(SG, NG, IB, 128, L)
        xt = np.ascontiguousarray(xt.transpose(0, 1, 3, 2, 4),
                                  dtype=ml_dtypes.bfloat16)
        xt = xt.reshape(SG * NG, 128, IB * L)
        in_maps.append({"xt": xt, "B1": B1, "B2": B2})
    return in_maps


def _unscramble(results, cfg=None):
    cfg = dict(CFG, **(cfg or {}))
    TS, OB = cfg["TS"], cfg["OB"]
    L = TS * N
    SG = R // L
    NCB = 32 // OB
    outs = []
    for i in range(N_CORES):
        Y = results[i]["Y"]      # [SG*NCB, 128, OB*L] bf16
        # [sg, cb, (hi', nu), j, rr] -> [sg, rr, hi', (cb, j, nu)]
        y = np.asarray(Y, dtype=np.float32).reshape(SG, NCB, 64, 2, OB, L)
        y = y.transpose(0, 5, 2, 1, 4, 3).reshape(R, DIM)
        outs.append(y)
    return np.concatenate(outs, axis=0).reshape(4, 4096, DIM).astype(np.float32)


def kernel(x, H, _trace=False, _cfg=None):
    nc = _build_bass(_cfg)
    in_maps = _prep_inputs(x, H, _cfg)
    res = run_bass_kernel_spmd(nc, in_maps, core_ids=list(range(N_CORES)),
                               trace=_trace)
    out = _unscramble(res.results, _cfg)
    if _trace:
        return out, res
    return out
